# revision 25
# baseline (speedup 1.0000x reference)
"""TRN2 Bass kernel for nn_ONOBlock (linear attention + MLPs + covariance whitening).

Sharding: data-parallel over batch, 1 batch element per core (B=8, n_cores=8).
Two launches with a host boundary for the [64,64] covariance all-reduce + Cholesky:
  fx_out = X_ @ (L^-T diag(softplus(mu)) L^-1) @ (X_^T fx)
so the per-token whitening matmul disappears and only cov crosses cores.

All heavy matmuls run as float32r (round-to-nearest-11-bit-mantissa, 1 cy/row,
measured 1.5e-4 rel err). LN gains fold into the following weights on the host;
zero biases are skipped at build time (rank-1 ones-matmul fallback if nonzero).
"""
import contextlib
import numpy as np

import bass_rust as _bass_rust
import concourse.bass as bass
import concourse.bacc as bacc
import concourse.tile as tile
from concourse import mybir
from concourse.hw_specs import get_activation_tables
from concourse.bass_utils import run_bass_kernel_spmd
from concourse.masks import make_identity

class _Bacc(bacc.Bacc):
    """Bacc with act-table selection steered to the combined ln+exp set.

    The stock pass resolves Ln->'natural_log' and Exp->'exp_and_others',
    reloading the ACT table between them (~1.3us each, every chunk).
    Masking those two sets forces both onto 'natural_log_exp_and_others'."""

    def insert_act_table_loads(self):
        has_activation = any(
            isinstance(i, mybir.InstActivation)
            for b in self.main_func.blocks
            for i in b.instructions
        )
        if not has_activation:
            return
        tabs = [
            (nm, (set() if nm in ("natural_log", "exp_and_others") else fs))
            for nm, fs in get_activation_tables(self.m.arch).items()
        ]
        _bass_rust.insert_act_table_loads(self, tabs)


F32 = mybir.dt.float32
F32R = mybir.dt.float32r
AF = mybir.ActivationFunctionType
ALU = mybir.AluOpType
AX = mybir.AxisListType

B, N, D, H, PSI = 8, 7225, 256, 8, 64
DH = D // H
DF = 4 * D
EPS = 1e-5
NP_ = 7232            # padded sequence: 56*128 + 64
NCH1 = 57             # pass-1 chunks (56 of 128 + 1 of 64)
NCH2 = 15             # pass-2 chunks (14 of 512 + 1 of 64)
CORES = list(range(8))


def _bcast(ap, parts):
    """Free-dim broadcast helper: [p, g] -> [p, g, parts] with 0-stride."""
    return bass.AP(tensor=ap.tensor, offset=ap.offset,
                   ap=[ap.ap[0], ap.ap[1], [0, parts]])


def _ln_stats(nc, pool, x_ap, w, mv_slot):
    """bn stats into mv_slot [w, 2] = (mean, var)."""
    stats = pool.tile([128, 6], F32, tag="ln_stats")
    nc.vector.bn_stats(out=stats[0:w], in_=x_ap)
    nc.vector.bn_aggr(out=mv_slot, in_=stats[0:w])


def _ln_rstd(nc, rstd_out, var_ap, eps_t):
    """rstd = exp(-0.5*ln(var+eps)); Ln and Exp share ACT func set 6 (no table switch)."""
    nc.scalar.activation(rstd_out, var_ap, AF.Ln, bias=eps_t)
    nc.scalar.activation(rstd_out, rstd_out, AF.Exp, scale=-0.5)


I32 = mybir.dt.int32


def _dve_rsqrt(nc, pool, var_ap, w, n, rstd_out, eps, magic):
    """rstd_out[0:w, 0:n] = 1/sqrt(var_ap + eps) entirely on DVE.

    Quake bit-trick init + 2 Newton steps; ~1e-5 rel err. Keeps the ACT
    engine free of Sqrt/Ln (which share no table set with Gelu)."""
    v4 = pool.tile([128, 4], F32, tag="rs_v")
    nc.vector.tensor_scalar(out=v4[0:w, 0:n], in0=var_ap, scalar1=float(eps),
                            scalar2=None, op0=ALU.add)
    sh = pool.tile([128, 4], I32, tag="rs_sh")
    nc.vector.tensor_scalar(out=sh[0:w, 0:n], in0=v4[0:w, 0:n].bitcast(I32),
                            scalar1=1, scalar2=None, op0=ALU.logical_shift_right)
    y = rstd_out
    nc.vector.tensor_tensor(out=y[0:w, 0:n].bitcast(I32), in0=magic[0:w, 0:n],
                            in1=sh[0:w, 0:n], op=ALU.subtract)
    t = pool.tile([128, 4], F32, tag="rs_t")
    for _ in range(2):
        nc.vector.tensor_tensor(out=t[0:w, 0:n], in0=y[0:w, 0:n], in1=y[0:w, 0:n], op=ALU.mult)
        nc.vector.tensor_tensor(out=t[0:w, 0:n], in0=t[0:w, 0:n], in1=v4[0:w, 0:n], op=ALU.mult)
        nc.vector.tensor_scalar(out=t[0:w, 0:n], in0=t[0:w, 0:n], scalar1=-0.5,
                                scalar2=1.5, op0=ALU.mult, op1=ALU.add)
        nc.vector.tensor_tensor(out=y[0:w, 0:n], in0=y[0:w, 0:n], in1=t[0:w, 0:n], op=ALU.mult)


def _ln_apply(nc, h_out, x_ap, mean_ap, rstd_ap, w):
    nc.vector.tensor_scalar(out=h_out[0:w], in0=x_ap, scalar1=mean_ap,
                            scalar2=rstd_ap, op0=ALU.subtract, op1=ALU.mult)


def _ln_ops(nc, pool, x_ap, w, h_out, eps_t):
    """Single-sub LayerNorm (gain/bias folded into weights)."""
    mv = pool.tile([128, 2], F32, tag="ln_mv")
    _ln_stats(nc, pool, x_ap, w, mv[0:w])
    rstd = pool.tile([128, 1], F32, tag="ln_rstd")
    _ln_rstd(nc, rstd[0:w], mv[0:w, 1:2], eps_t[0:w])
    _ln_apply(nc, h_out, x_ap, mv[0:w, 0:1], rstd[0:w], w)


def _transpose_pair(nc, ptr_pool, ident_m, src, w, dst_ap, copy_eng):
    """PE-transpose src[0:w, 0:128] and src[0:w, 128:256] into one psum tile,
    then a single copy to dst_ap ([128, 2, w] view). ident_m matches src dtype."""
    dt_ = src.dtype
    pt = ptr_pool.tile([128, 256], dt_, tag="tr", name="pt")
    for dc in range(2):
        nc.tensor.matmul(pt[:, dc * w:(dc + 1) * w], src[0:w, dc * 128:(dc + 1) * 128],
                         ident_m[0:w, 0:w], is_transpose=True,
                         skip_group_check=(dc == 1))
    copy_eng(dst_ap, pt[:, 0:2 * w].rearrange("p (c w) -> p c w", c=2))


def build_launch1(flags):
    nc = _Bacc(None)
    # ---- I/O ----
    x_d = nc.dram_tensor("x", [NP_, D], F32, kind="ExternalInput")
    fx_d = nc.dram_tensor("fx", [NP_, D], F32R, kind="ExternalInput")
    wqkv_d = nc.dram_tensor("wqkv", [D, 3 * D], F32R, kind="ExternalInput")
    wo_d = nc.dram_tensor("wo", [D, D], F32R, kind="ExternalInput")
    w1_d = nc.dram_tensor("w1", [D, DF], F32R, kind="ExternalInput")
    w2_d = nc.dram_tensor("w2", [DF, D], F32R, kind="ExternalInput")
    p1_d = nc.dram_tensor("p1", [D, D], F32R, kind="ExternalInput")
    p2_d = nc.dram_tensor("p2", [D, PSI], F32R, kind="ExternalInput")
    cmask_d = nc.dram_tensor("cmask", [D, D], F32, kind="ExternalInput")
    ib1_d = nc.dram_tensor("ib1", [DF], F32, kind="ExternalInput")
    ip1_d = nc.dram_tensor("ip1", [D], F32, kind="ExternalInput")
    ipb2_d = nc.dram_tensor("ipb2", [PSI], F32, kind="ExternalInput")
    if flags["bqkv"]:
        bqkv_d = nc.dram_tensor("bqkv", [1, 3 * D], F32R, kind="ExternalInput")
    if flags["bo"]:
        bo_d = nc.dram_tensor("bo", [1, D], F32R, kind="ExternalInput")
    if flags["b2"]:
        b2_d = nc.dram_tensor("b2", [1, D], F32R, kind="ExternalInput")

    x2o_d = nc.dram_tensor("x2o", [NP_, D], F32, kind="ExternalOutput")
    xt_d = nc.dram_tensor("xt", [PSI, NP_], F32, kind="ExternalOutput")
    cov_d = nc.dram_tensor("cov", [PSI, PSI], F32, kind="ExternalOutput")
    c2p_d = nc.dram_tensor("c2p", [PSI, D], F32, kind="ExternalOutput")

    with tile.TileContext(nc) as tc, contextlib.ExitStack() as top:
        wp = top.enter_context(tc.tile_pool(name="wp", bufs=1))
        # ---- resident weights/constants ----
        wqkv = wp.tile([128, 2, 3 * D], F32R)
        nc.sync.dma_start(out=wqkv, in_=wqkv_d.rearrange("(c p) e -> p c e", p=128))
        wo = wp.tile([128, 2, D], F32R)
        nc.sync.dma_start(out=wo, in_=wo_d.rearrange("(c p) e -> p c e", p=128))
        w1 = wp.tile([128, 2, DF], F32R)
        nc.sync.dma_start(out=w1, in_=w1_d.rearrange("(c p) e -> p c e", p=128))
        w2 = wp.tile([128, 8, D], F32R)
        nc.sync.dma_start(out=w2, in_=w2_d.rearrange("(c p) e -> p c e", p=128))
        p1 = wp.tile([128, 2, D], F32R)
        nc.sync.dma_start(out=p1, in_=p1_d.rearrange("(c p) e -> p c e", p=128))
        p2 = wp.tile([128, 2, PSI], F32R)
        nc.sync.dma_start(out=p2, in_=p2_d.rearrange("(c p) e -> p c e", p=128))
        cmask = wp.tile([128, 2, D], F32)
        nc.sync.dma_start(out=cmask, in_=cmask_d.rearrange("(c p) e -> p c e", p=128))
        ib1 = wp.tile([128, 8], F32)
        nc.sync.dma_start(out=ib1, in_=ib1_d.rearrange("(a p) -> p a", p=128))
        ip1 = wp.tile([128, 2], F32)
        nc.sync.dma_start(out=ip1, in_=ip1_d.rearrange("(a p) -> p a", p=128))
        ipb2 = wp.tile([64, 1], F32)
        nc.sync.dma_start(out=ipb2, in_=ipb2_d.rearrange("(p a) -> p a", a=1))
        if flags["bqkv"]:
            bqkv = wp.tile([1, 3 * D], F32R)
            nc.sync.dma_start(out=bqkv, in_=bqkv_d[:])
        if flags["bo"]:
            bo = wp.tile([1, D], F32R)
            nc.sync.dma_start(out=bo, in_=bo_d[:])
        if flags["b2"]:
            b2 = wp.tile([1, D], F32R)
            nc.sync.dma_start(out=b2, in_=b2_d[:])

        eps_t = wp.tile([128, 1], F32)
        nc.vector.memset(eps_t, EPS)
        magic = wp.tile([128, 4], I32)
        nc.vector.memset(magic, 0x5F3759DF)
        ident = wp.tile([128, 128], F32)
        make_identity(nc, ident)
        ident_r = wp.tile([128, 128], F32R)
        nc.vector.tensor_copy(ident_r, ident)
        ones_f = wp.tile([128, 16], F32)
        nc.vector.memset(ones_f, 1.0)
        ones_col = wp.tile([128, 1], F32R)
        nc.vector.tensor_copy(ones_col, ones_f[:, 0:1])
        onesc_r = wp.tile([128, 2], F32R)
        nc.vector.tensor_copy(onesc_r, ones_f[:, 0:2])
        zero_f = wp.tile([128, 16], F32)
        nc.vector.memset(zero_f, 0.0)

        qT = wp.tile([128, 2, NP_], F32R)      # q softmax'd, transposed, resident
        C_sb = wp.tile([128, 2, D], F32R)      # masked/scaled context matrix
        CW_sb = wp.tile([128, 2, D], F32R)     # C @ Wo

        # ================= PASS 1 =================
        with contextlib.ExitStack() as s1:
            sb = s1.enter_context(tc.tile_pool(name="p1sb", bufs=3))
            pctx = s1.enter_context(tc.tile_pool(name="pctx", bufs=1, space="PSUM"))
            pqk = s1.enter_context(tc.tile_pool(name="pqk", bufs=2, space="PSUM"))
            pv = s1.enter_context(tc.tile_pool(name="pv", bufs=1, space="PSUM"))
            ptr = s1.enter_context(tc.tile_pool(name="ptr", bufs=3, space="PSUM"))

            ctx_ps = [pctx.tile([128, 264], F32, tag=f"ctx{dc}", name=f"ctx_ps{dc}")
                      for dc in range(2)]

            def p1dim(c):
                t0 = c * 128
                w = 128 if c < NCH1 - 1 else 64
                return t0, w

            def p1_front(c):
                t0, w = p1dim(c)
                x_sb = sb.tile([128, D], F32, tag="x_in", name="x_sb")
                nc.sync.dma_start(out=x_sb[0:w], in_=x_d[t0:t0 + w, :])
                h0 = sb.tile([128, D], F32R, tag="h0", name="h0")
                _ln_ops(nc, sb, x_sb[0:w], w, h0, eps_t)
                h0T = sb.tile([128, 2, 128], F32R, tag="h0T", name="h0T")
                _transpose_pair(nc, ptr, ident_r, h0, w, h0T[:, :, 0:w],
                                lambda d_, s_: nc.vector.tensor_copy(d_, s_))
                return h0T

            def p1_qkv(c, h0T):
                t0, w = p1dim(c)
                ps_qk = pqk.tile([128, 2 * D], F32, tag="qk", name="ps_qk")
                ps_v = pv.tile([128, D], F32, tag="v", name="ps_v")
                for i in range(2):
                    for dc in range(2):
                        nc.tensor.matmul(ps_qk[0:w, i * D:(i + 1) * D], h0T[:, dc, 0:w],
                                         wqkv[:, dc, i * D:(i + 1) * D],
                                         start=(dc == 0 and i == 0),
                                         stop=(dc == 1 and not flags["bqkv"]),
                                         skip_group_check=(i == 1))
                    if flags["bqkv"]:
                        nc.tensor.matmul(ps_qk[0:w, i * D:(i + 1) * D],
                                         ones_col[0:1, 0:1].broadcast_to([1, w]),
                                         bqkv[:, i * D:(i + 1) * D], start=False, stop=True,
                                         skip_group_check=True)
                for dc in range(2):
                    nc.tensor.matmul(ps_v[0:w], h0T[:, dc, 0:w],
                                     wqkv[:, dc, 2 * D:3 * D],
                                     start=(dc == 0), stop=(dc == 1 and not flags["bqkv"]))
                if flags["bqkv"]:
                    nc.tensor.matmul(ps_v[0:w], ones_col[0:1, 0:1].broadcast_to([1, w]),
                                     bqkv[:, 2 * D:3 * D], start=False, stop=True)
                return ps_qk, ps_v

            def p1_back(c, ps_qk, ps_v):
                t0, w = p1dim(c)
                eqk = sb.tile([128, 2 * D], F32R, tag="eqk", name="eqk")
                nc.scalar.activation(eqk[0:w], ps_qk[0:w], AF.Exp)
                eq = eqk[:, 0:D]
                ek = eqk[:, D:2 * D]
                qs = sb.tile([128, 8], F32, tag="qs", name="qs")
                nc.vector.reduce_sum(out=qs[0:w], in_=eq[0:w].rearrange("p (g s) -> p g s", g=8), axis=AX.X)
                nc.vector.reciprocal(qs[0:w], qs[0:w])
                q_sm = sb.tile([128, D], F32R, tag="q_sm", name="q_sm")
                nc.gpsimd.tensor_tensor(out=q_sm[0:w].rearrange("p (g s) -> p g s", g=8),
                                        in0=eq[0:w].rearrange("p (g s) -> p g s", g=8),
                                        in1=_bcast(qs[0:w], 32), op=ALU.mult)
                _transpose_pair(nc, ptr, ident_r, q_sm, w, qT[:, :, t0:t0 + w],
                                lambda d_, s_: nc.scalar.activation(d_, s_, AF.Copy))

                v_sb = sb.tile([128, D], F32R, tag="v_sb", name="v_sb")
                nc.scalar.activation(v_sb[0:w], ps_v[0:w], AF.Copy)
                kv = w if c < NCH1 - 1 else N - t0
                for dc in range(2):
                    nc.tensor.matmul(ctx_ps[dc][:, 0:D], ek[0:kv, dc * 128:(dc + 1) * 128],
                                     v_sb[0:kv], start=(c == 0), stop=(c == NCH1 - 1))
                    nc.tensor.matmul(ctx_ps[dc][:, 256:258], ek[0:kv, dc * 128:(dc + 1) * 128],
                                     onesc_r[0:kv], start=False, stop=(c == NCH1 - 1),
                                     skip_group_check=True)

            h0T_c = p1_front(0)
            for c in range(NCH1):
                qkv = p1_qkv(c, h0T_c)
                h0T_c = p1_front(c + 1) if c + 1 < NCH1 else None
                p1_back(c, *qkv)

            for dc in range(2):
                nc.vector.tensor_copy(qT[:, dc, N:NP_], zero_f[:, 0:NP_ - N])

            # ---- build C = blockdiag_mask * DH^-0.5 * diag(1/Z) @ ctx ----
            for dc in range(2):
                zr = sb.tile([128, 1], F32, tag="zr")
                nc.vector.reciprocal(zr, ctx_ps[dc][:, 256:257])
                ct = sb.tile([128, D], F32, tag="ct")
                nc.vector.tensor_scalar(out=ct, in0=ctx_ps[dc][:, 0:D], scalar1=zr,
                                        scalar2=None, op0=ALU.mult)
                nc.vector.tensor_tensor(out=C_sb[:, dc, :], in0=ct, in1=cmask[:, dc, :], op=ALU.mult)
            # CT = C^T, then CW = C @ Wo  (x1 = q_sm @ C @ Wo, associativity)
            CT_sb = wp.tile([128, 2, D], F32R)
            for dc in range(2):
                _transpose_pair(nc, ptr, ident_r, C_sb[:, dc, :], 128, CT_sb[:, :, dc * 128:(dc + 1) * 128].rearrange("p c w -> p c w"),
                                lambda d_, s_: nc.vector.tensor_copy(d_, s_))
            for m in range(2):
                cwps = pqk.tile([128, 2 * D], F32, tag="qk", name="cwps")
                for ec in range(2):
                    nc.tensor.matmul(cwps[:, 0:D], CT_sb[:, ec, m * 128:(m + 1) * 128],
                                     wo[:, ec, :], start=(ec == 0), stop=(ec == 1))
                nc.vector.tensor_copy(CW_sb[:, m, :], cwps[:, 0:D])

        # ================= PASS 2 =================
        with contextlib.ExitStack() as s2:
            sb = s2.enter_context(tc.tile_pool(name="p2sb", bufs=2))
            sb3 = s2.enter_context(tc.tile_pool(name="p2sb3", bufs=3))
            pcc = s2.enter_context(tc.tile_pool(name="pcc", bufs=1, space="PSUM"))
            pbig = s2.enter_context(tc.tile_pool(name="pbig", bufs=3, space="PSUM"))
            px2 = s2.enter_context(tc.tile_pool(name="px2", bufs=1, space="PSUM"))
            ptr = s2.enter_context(tc.tile_pool(name="ptr2", bufs=2, space="PSUM"))

            cc_ps = pcc.tile([64, 320], F32)

            def chdim(C):
                T0 = C * 512
                T = 512 if C < NCH2 - 1 else 64
                nsub = T // 128 if C < NCH2 - 1 else 1
                sw = 128 if C < NCH2 - 1 else 64
                return T0, T, nsub, sw

            def front(C):
                """attention apply + residual + LN2 -> h2T for chunk C."""
                T0, T, nsub, sw = chdim(C)
                x1_sb = sb.tile([128, 4, D], F32, tag="x1", name="x1_sb")
                h2T = sb.tile([128, 2, 512], F32R, tag="h2T", name="h2T")
                mv4 = sb.tile([128, 4, 2], F32, tag="mv4", name="mv4")
                rstd4 = sb.tile([128, 4], F32, tag="rstd4", name="rstd4")
                for s in range(nsub):
                    t0 = T0 + s * 128
                    xps = pbig.tile([128, 512], F32, tag="big", name="xps")
                    for dc in range(2):
                        nc.tensor.matmul(xps[0:sw, 0:D], qT[:, dc, t0:t0 + sw],
                                         CW_sb[:, dc, :],
                                         start=(dc == 0), stop=(dc == 1 and not flags["bo"]))
                    if flags["bo"]:
                        nc.tensor.matmul(xps[0:sw, 0:D], ones_col[0:1, 0:1].broadcast_to([1, sw]),
                                         bo[:], start=False, stop=True)
                    x_in = sb3.tile([128, D], F32, tag="x_in2", name="x_in")
                    nc.sync.dma_start(out=x_in[0:sw], in_=x_d[t0:t0 + sw, :])
                    nc.vector.tensor_tensor(out=x1_sb[0:sw, s, :], in0=xps[0:sw, 0:D],
                                            in1=x_in[0:sw], op=ALU.add)
                    _ln_stats(nc, sb3, x1_sb[0:sw, s, :], sw, mv4[0:sw, s, :])
                    pass
                _dve_rsqrt(nc, sb3, mv4[0:sw, 0:nsub, 1:2], sw, nsub, rstd4, EPS, magic)
                for s in range(nsub):
                    h2 = sb3.tile([128, D], F32R, tag="h2", name="h2")
                    _ln_apply(nc, h2, x1_sb[0:sw, s, :], mv4[0:sw, s, 0:1],
                              rstd4[0:sw, s:s + 1], sw)
                    _transpose_pair(nc, ptr, ident_r, h2, sw,
                                    h2T[:, :, s * 128:s * 128 + sw],
                                    lambda d_, s_: nc.vector.tensor_copy(d_, s_))
                return x1_sb, h2T

            def back_mlp(C, st):
                """u/gelu/x2-accumulate for chunk C."""
                T0, T, nsub, sw = chdim(C)
                x1_sb, h2T = st
                x2acc = px2.tile([128, 4, D], F32, tag="x2acc", name="x2acc")
                for fs in range(8):
                    ups = pbig.tile([128, 512], F32, tag="big", name="ups")
                    for dc in range(2):
                        nc.tensor.matmul(ups[:, 0:T], w1[:, dc, fs * 128:(fs + 1) * 128],
                                         h2T[:, dc, 0:T], start=(dc == 0), stop=(dc == 1))
                    uT = sb3.tile([128, 512], F32R, tag="uT", name="uT")
                    nc.scalar.activation(uT[:, 0:T], ups[:, 0:T], AF.Gelu,
                                         bias=ib1[:, fs:fs + 1])
                    for s in range(nsub):
                        nc.tensor.matmul(x2acc[0:sw, s, :], uT[:, s * 128:s * 128 + sw],
                                         w2[:, fs, :],
                                         start=(fs == 0 and s % 2 == 0),
                                         stop=(fs == 7 and not flags["b2"]),
                                         skip_group_check=(fs > 0 or s % 2 == 1))
                if flags["b2"]:
                    for s in range(nsub):
                        nc.tensor.matmul(x2acc[0:sw, s, :], ones_col[0:1, 0:1].broadcast_to([1, sw]),
                                         b2[:], start=False, stop=True, skip_group_check=True)
                return x2acc

            def back_tail(C, st, x2acc):
                T0, T, nsub, sw = chdim(C)
                x1_sb, h2T = st
                x2T = sb.tile([128, 2, 512], F32R, tag="x2T", name="x2T")
                for s in range(nsub):
                    t0 = T0 + s * 128
                    x2_sb = sb3.tile([128, D], F32, tag="x2_sb", name="x2_sb")
                    nc.vector.tensor_tensor(out=x2_sb[0:sw], in0=x2acc[0:sw, s, :],
                                            in1=x1_sb[0:sw, s, :], op=ALU.add)
                    nc.sync.dma_start(out=x2o_d[t0:t0 + sw, :], in_=x2_sb[0:sw])
                    _transpose_pair(nc, ptr, ident, x2_sb, sw,
                                    x2T[:, :, s * 128:s * 128 + sw],
                                    lambda d_, s_: nc.scalar.activation(d_, s_, AF.Copy))

                pT = sb.tile([128, 2, 512], F32R, tag="pT", name="pT")
                for pc in range(2):
                    pps = pbig.tile([128, 512], F32, tag="big", name="pps")
                    for dc in range(2):
                        nc.tensor.matmul(pps[:, 0:T], p1[:, dc, pc * 128:(pc + 1) * 128],
                                         x2T[:, dc, 0:T], start=(dc == 0), stop=(dc == 1))
                    nc.scalar.activation(pT[:, pc, 0:T], pps[:, 0:T], AF.Gelu,
                                         bias=ip1[:, pc:pc + 1])
                xtps = pbig.tile([128, 512], F32, tag="big", name="xtps")
                for pc in range(2):
                    nc.tensor.matmul(xtps[0:64, 0:T], p2[:, pc, :], pT[:, pc, 0:T],
                                     start=(pc == 0), stop=(pc == 1))
                xT_sb = sb.tile([64, 512], F32R, tag="xT_sb", name="xT_sb")
                nc.scalar.activation(xT_sb[:, 0:T], xtps[0:64, 0:T], AF.Identity,
                                     bias=ipb2[:, 0:1])
                nc.sync.dma_start(out=xt_d[:, T0:T0 + T], in_=xT_sb[:, 0:T].bitcast(F32))

                for s in range(nsub):
                    t0 = T0 + s * 128
                    vv = min(sw, N - t0)
                    xc = sb3.tile([128, 320], F32R, tag="xc", name="xc")
                    xtr = ptr.tile([128, 128], F32R, tag="tr", name="xtr")
                    nc.tensor.transpose(xtr[0:sw, 0:64], xT_sb[:, s * 128:s * 128 + sw],
                                        ident_r[0:64, 0:64])
                    if vv < sw and flags.get("anybias"):
                        nc.vector.tensor_copy(xc[0:sw, :],
                                              _bcast(zero_f[0:sw, 0:1], 320).rearrange("p a b -> p (a b)"))
                        nc.vector.tensor_copy(xc[0:vv, 0:64], xtr[0:vv, 0:64])
                    else:
                        nc.vector.tensor_copy(xc[0:sw, 0:64], xtr[0:sw, 0:64])
                    nc.sync.dma_start(out=xc[0:sw, 64:320], in_=fx_d[t0:t0 + sw, :])
                    nc.tensor.matmul(cc_ps, xc[0:sw, 0:64], xc[0:sw, :],
                                     start=(C == 0 and s == 0),
                                     stop=(C == NCH2 - 1 and s == nsub - 1))

            # software pipeline: front(C+1) emitted between MLP(C) and tail(C)
            st = front(0)
            for C in range(NCH2):
                x2acc = back_mlp(C, st)
                back_tail(C, st, x2acc)
                st = front(C + 1) if C + 1 < NCH2 else None

            cc_sb = sb.tile([64, 320], F32, tag="cc_sb")
            nc.vector.tensor_copy(cc_sb, cc_ps)
            nc.sync.dma_start(out=cov_d[:], in_=cc_sb[:, 0:64])
            nc.sync.dma_start(out=c2p_d[:], in_=cc_sb[:, 64:320])

    nc.finalize()
    return nc


def build_launch2(flags):
    nc = _Bacc(None)
    xt_d = nc.dram_tensor("xt", [PSI, NP_], F32R, kind="ExternalInput")
    c2pp_d = nc.dram_tensor("c2pp", [PSI, D], F32R, kind="ExternalInput")
    m1_d = nc.dram_tensor("m1", [D, DF], F32R, kind="ExternalInput")
    m2_d = nc.dram_tensor("m2", [DF, D], F32R, kind="ExternalInput")
    ib2_d = nc.dram_tensor("ib2", [DF], F32, kind="ExternalInput")
    if flags["mb2"]:
        mb2_d = nc.dram_tensor("mb2", [1, D], F32R, kind="ExternalInput")
    fxo_d = nc.dram_tensor("fxo", [NP_, D], F32, kind="ExternalOutput")

    with tile.TileContext(nc) as tc, contextlib.ExitStack() as top:
        wp = top.enter_context(tc.tile_pool(name="wp", bufs=1))
        xt_all = wp.tile([64, NP_], F32R)
        nc.sync.dma_start(out=xt_all, in_=xt_d[:])
        c2pp = wp.tile([64, D], F32R)
        nc.sync.dma_start(out=c2pp, in_=c2pp_d[:])
        m1 = wp.tile([128, 2, DF], F32R)
        nc.sync.dma_start(out=m1, in_=m1_d.rearrange("(c p) e -> p c e", p=128))
        m2 = wp.tile([128, 8, D], F32R)
        nc.sync.dma_start(out=m2, in_=m2_d.rearrange("(c p) e -> p c e", p=128))
        ib2 = wp.tile([128, 8], F32)
        nc.sync.dma_start(out=ib2, in_=ib2_d.rearrange("(a p) -> p a", p=128))
        if flags["mb2"]:
            mb2 = wp.tile([1, D], F32R)
            nc.sync.dma_start(out=mb2, in_=mb2_d[:])
            ones_f = wp.tile([128, 1], F32)
            nc.vector.memset(ones_f, 1.0)
            ones_col = wp.tile([128, 1], F32R)
            nc.vector.tensor_copy(ones_col, ones_f)
        eps_t = wp.tile([128, 1], F32)
        nc.vector.memset(eps_t, EPS)
        magic = wp.tile([128, 4], I32)
        nc.vector.memset(magic, 0x5F3759DF)
        ident = wp.tile([128, 128], F32)
        make_identity(nc, ident)
        ident_r = wp.tile([128, 128], F32R)
        nc.vector.tensor_copy(ident_r, ident)

        with contextlib.ExitStack() as s1:
            sb = s1.enter_context(tc.tile_pool(name="sb", bufs=2))
            sb3 = s1.enter_context(tc.tile_pool(name="sb3", bufs=3))
            pbig = s1.enter_context(tc.tile_pool(name="pbig", bufs=2, space="PSUM"))
            pmid = s1.enter_context(tc.tile_pool(name="pmid", bufs=2, space="PSUM"))
            pacc = s1.enter_context(tc.tile_pool(name="pacc", bufs=1, space="PSUM"))
            ptr = s1.enter_context(tc.tile_pool(name="ptr", bufs=2, space="PSUM"))

            def chdim(C):
                T0 = C * 512
                T = 512 if C < NCH2 - 1 else 64
                nsub = T // 128 if C < NCH2 - 1 else 1
                sw = 128 if C < NCH2 - 1 else 64
                return T0, T, nsub, sw

            def front(C):
                T0, T, nsub, sw = chdim(C)
                h3T = sb.tile([128, 2, 512], F32R, tag="h3T", name="h3T")
                mv4 = sb.tile([128, 4, 2], F32, tag="mv4", name="mv4")
                rstd4 = sb.tile([128, 4], F32, tag="rstd4", name="rstd4")
                fxu4 = sb.tile([128, 4, D], F32, tag="fxu4", name="fxu4")
                for s in range(nsub):
                    t0 = T0 + s * 128
                    fps = pmid.tile([128, D], F32, tag="fxu", name="fps")
                    nc.tensor.matmul(fps[0:sw], xt_all[:, t0:t0 + sw], c2pp[:],
                                     start=True, stop=True)
                    nc.vector.tensor_copy(fxu4[0:sw, s, :], fps[0:sw])
                    _ln_stats(nc, sb3, fxu4[0:sw, s, :], sw, mv4[0:sw, s, :])
                _dve_rsqrt(nc, sb3, mv4[0:sw, 0:nsub, 1:2], sw, nsub, rstd4, EPS, magic)
                for s in range(nsub):
                    h3 = sb3.tile([128, D], F32R, tag="h3", name="h3")
                    _ln_apply(nc, h3, fxu4[0:sw, s, :], mv4[0:sw, s, 0:1],
                              rstd4[0:sw, s:s + 1], sw)
                    _transpose_pair(nc, ptr, ident_r, h3, sw,
                                    h3T[:, :, s * 128:s * 128 + sw],
                                    lambda d_, s_: nc.vector.tensor_copy(d_, s_))
                return h3T

            def back(C, h3T):
                T0, T, nsub, sw = chdim(C)
                facc = pacc.tile([128, 4, D], F32, tag="facc", name="facc")
                for fs in range(8):
                    ups = pbig.tile([128, 512], F32, tag="big", name="ups")
                    for dc in range(2):
                        nc.tensor.matmul(ups[:, 0:T], m1[:, dc, fs * 128:(fs + 1) * 128],
                                         h3T[:, dc, 0:T], start=(dc == 0), stop=(dc == 1))
                    uT = sb3.tile([128, 512], F32R, tag="uT", name="uT")
                    nc.scalar.activation(uT[:, 0:T], ups[:, 0:T], AF.Gelu,
                                         bias=ib2[:, fs:fs + 1])
                    for s in range(nsub):
                        nc.tensor.matmul(facc[0:sw, s, :], uT[:, s * 128:s * 128 + sw],
                                         m2[:, fs, :],
                                         start=(fs == 0 and s % 2 == 0),
                                         stop=(fs == 7 and not flags["mb2"]),
                                         skip_group_check=(fs > 0 or s % 2 == 1))
                if flags["mb2"]:
                    for s in range(nsub):
                        nc.tensor.matmul(facc[0:sw, s, :], ones_col[0:1, 0:1].broadcast_to([1, sw]),
                                         mb2[:], start=False, stop=True, skip_group_check=True)
                for s in range(nsub):
                    t0 = T0 + s * 128
                    fo = sb3.tile([128, D], F32, tag="fo", name="fo")
                    nc.vector.tensor_copy(fo[0:sw], facc[0:sw, s, :])
                    nc.sync.dma_start(out=fxo_d[t0:t0 + sw, :], in_=fo[0:sw])

            h3T_c = front(0)
            for C in range(NCH2):
                bk = h3T_c
                h3T_c = front(C + 1) if C + 1 < NCH2 else None
                back(C, bk)

    nc.finalize()
    return nc


_NC_CACHE = {}


def _get_nc(which, flags):
    key = (which, tuple(sorted(flags.items())))
    if key not in _NC_CACHE:
        _NC_CACHE[key] = build_launch1(flags) if which == 1 else build_launch2(flags)
    return _NC_CACHE[key]


def kernel(**inputs):
    inp = {k: np.ascontiguousarray(np.asarray(v)) for k, v in inputs.items()}
    x, fx = inp["x"], inp["fx"]
    f64 = lambda k: inp[k].astype(np.float64)

    # ---- host-side weight folding (LN gains into following weights) ----
    g1, b1 = f64("ln1_g"), f64("ln1_b")
    g2, b2 = f64("ln2_g"), f64("ln2_b")
    g3, b3 = f64("ln3_g"), f64("ln3_b")
    Wq, Wk, Wv = f64("Wq"), f64("Wk"), f64("Wv")
    wqkv = np.concatenate([g1[:, None] * Wq, g1[:, None] * Wk, g1[:, None] * Wv],
                          axis=1).astype(np.float32)
    bqkv = np.concatenate([b1 @ Wq, b1 @ Wk, b1 @ Wv]).astype(np.float32)[None, :]
    w1 = (g2[:, None] * f64("mlp_W1")).astype(np.float32)
    ib1 = (b2 @ f64("mlp_W1") + f64("mlp_b1")).astype(np.float32)
    m1 = (g3[:, None] * f64("mlp2_W1")).astype(np.float32)
    ib2 = (b3 @ f64("mlp2_W1") + f64("mlp2_b1")).astype(np.float32)
    cmask = np.zeros((D, D), np.float32)
    for h in range(H):
        cmask[h * DH:(h + 1) * DH, h * DH:(h + 1) * DH] = DH ** -0.5

    flags1 = {"bqkv": bool(np.any(bqkv)), "bo": bool(np.any(inp["bo"])),
              "b2": bool(np.any(inp["mlp_b2"]))}
    flags1["anybias"] = any(flags1.values()) or bool(np.any(ib1)) or bool(np.any(inp["proj_b1"])) or bool(np.any(inp["proj_b2"]))
    xp = np.zeros((B, NP_, D), np.float32); xp[:, :N] = x
    fxp = np.zeros((B, NP_, D), np.float32); fxp[:, :N] = fx

    flags2 = {"mb2": bool(np.any(inp["mlp2_b2"]))}

    common1 = {
        "wqkv": wqkv, "wo": inp["Wo"], "w1": w1, "w2": inp["mlp_W2"],
        "p1": inp["proj_W1"], "p2": inp["proj_W2"], "cmask": cmask,
        "ib1": ib1, "ip1": inp["proj_b1"], "ipb2": inp["proj_b2"],
    }
    if flags1["bqkv"]:
        common1["bqkv"] = bqkv
    if flags1["bo"]:
        common1["bo"] = inp["bo"][None, :].astype(np.float32)
    if flags1["b2"]:
        common1["b2"] = inp["mlp_b2"][None, :].astype(np.float32)

    nc1 = _get_nc(1, flags1)
    in_maps1 = [dict(common1, x=xp[b], fx=fxp[b]) for b in range(B)]
    res1 = run_bass_kernel_spmd(nc1, in_maps1, CORES).results
    res1 = [{k: np.asarray(v) for k, v in r.items()} for r in res1]

    # ---- host boundary: cov all-reduce + Cholesky + M fold ----
    cov = sum(r["cov"].astype(np.float64) for r in res1) / (B * N)
    L = np.linalg.cholesky(cov)
    Linv = np.linalg.inv(L)
    sp_mu = np.log1p(np.exp(inp["mu"].astype(np.float64)))
    M = Linv.T @ (sp_mu[:, None] * Linv)

    common2 = {"m1": m1, "m2": inp["mlp2_W2"], "ib2": ib2}
    if flags2["mb2"]:
        common2["mb2"] = inp["mlp2_b2"][None, :].astype(np.float32)
    nc2 = _get_nc(2, flags2)
    in_maps2 = [dict(common2, xt=res1[b]["xt"],
                     c2pp=(M @ res1[b]["c2p"].astype(np.float64)).astype(np.float32))
                for b in range(B)]
    res2 = run_bass_kernel_spmd(nc2, in_maps2, CORES).results
    res2 = [{k: np.asarray(v) for k, v in r.items()} for r in res2]

    x_out = np.stack([res1[b]["x2o"][:N] for b in range(B)]).astype(np.float32)
    fx_out = np.stack([res2[b]["fxo"][:N] for b in range(B)]).astype(np.float32)
    return x_out, fx_out
